# revision 53
# baseline (speedup 1.0000x reference)
"""Trainium2 Bass kernel for the SNN (snntorch Leaky, subtract-reset) forward.

Reference computation:
    cur1 = x @ W1.T + b1                      # [B, 100], static across steps
    25 steps of:  mem1 = 0.95*mem1 + cur1 - H(prev mem1 - 1)
                  spk1 = H(mem1 - 1);  cur2 = spk1 @ W2.T + b2
                  mem2 = 0.95*mem2 + cur2 - H(prev mem2 - 1)
    returns mem2 per step: [25, B, 2]

Device algorithm (per core, batch shard 8192, hidden on partitions):
  Let A = cur1/(1-beta), z = mem1 - A, theta = 1 - A.  Track
      V_t = beta^-t * (z_t - theta)
  so the spike test is the CONSTANT-threshold compare  spk_t = H(V_t):
      V_0 = -1 (uniform);  V_t = V_{t-1} - beta^-t spk_{t-1} - d_t theta,
      d_t = beta^-t - beta^-(t-1).
  The drift d_t*theta = d_t - d_t*(Mx x + Mb) is linear in x (9 features), so
  ONE accumulating PSUM matmul per step applies spike-subtract AND drift AND
  the fc2 readout: moving tile rows = [ones; x(9); pad; spk(100)], stationary
  [128, 128] = 28 cur2-slot cols + 100 V cols.  Spikes are computed from V by
  three engines in parallel on column slices: ScalarE Sign (+-1 coded
  stationaries), DVE and GPSIMD is_gt (0/1 coded).  Every (engine x parity)
  slice owns PRIVATE moving/PSUM/staging tiles -- the tile framework
  serializes cross-engine access to a shared tile even on disjoint ranges.
  cur2_t accumulates into a rotating set of 14 PSUM row-pairs; snapshots after
  step 15/26 + differencing recover all 25, then PE-transpose to batch-major
  and a fused 2-op/step recurrence (DVE/GPSIMD halves) produces mem2.
"""

import numpy as np

BETA = 0.95
T = 25
NI, NH, NO = 9, 100, 2
B = 65536
NCORES = 8
SH = B // NCORES          # batch shard per core
f32 = np.float32

W = 2048                  # round width (columns per engine-group of tiles)
NSLOT = 14                # rotating cur2 row-pair slots
SROW = 28                 # first spike row / number of slot rows
ENAMES = ("act", "dve")
ENC = (0, 1)              # stationary coding per engine: 0 = sign, 1 = 0/1
# per-parity engine slices (base, width) -- PSUM tiles are bank (512 col)
# quantized.  GPSIMD cannot touch PSUM (walrus rule), so only ScalarE and
# DVE read V; GPSIMD gets the SBUF-side work (staging subtract, mem2 half).
SL = [[(0, 1024), (1024, 1024)],
      [(0, 1024), (1024, 1024)]]

_CACHE = {}
_LAST_RESULT = None       # test.py pokes at these for its timing harness
_LAST_IN_MAPS = None


def _build_nop_nc():
    """Minimal kernel (one tiny DMA round-trip) for dispatch-overhead baseline."""
    import concourse.bass as bass
    import concourse.tile as tile
    from concourse import bacc, mybir
    f32d = mybir.dt.float32
    nc = bacc.Bacc("TRN2", target_bir_lowering=False, debug=False,
                   num_devices=NCORES)
    i_d = nc.dram_tensor("nin", [1, 128], f32d, kind="ExternalInput").ap()
    o_d = nc.dram_tensor("nout", [1, 128], f32d, kind="ExternalOutput").ap()
    with tile.TileContext(nc) as tc:
        with tc.tile_pool(name="sb", bufs=1) as sb:
            tl = sb.tile([1, 128], f32d)
            nc.sync.dma_start(tl[:], i_d[:])
            nc.sync.dma_start(o_d[:], tl[:])
    nc.compile()
    return nc


def host_stationaries(W1, b1, W2, b2):
    """[128, 52*128] f32 stationary bank: index (t-1)*2+enc, t = 1..26.

    Rows: 0:100 = spikes, 100:109 = x features, 109 = ones, 110:128 = pad.
    Cols: 0:100 V columns, 100:128 cur2 slots (col 100+2p+o).
    """
    inv = 1.0 / (1.0 - np.float64(BETA))
    Mx = W1.astype(np.float64) * inv          # [100, 9]
    Mb = b1.astype(np.float64) * inv          # [100]
    W2d = W2.astype(np.float64)
    b2d = b2.astype(np.float64)
    bi = 1.0 / np.float64(BETA)
    bp = {t: bi ** t for t in range(0, T + 2)}
    d = {t: bp[t] - bp[t - 1] for t in range(1, T + 2)}

    st = np.zeros((128, 52 * 128), np.float64)
    for t in range(1, T + 2):
        for enc in (0, 1):
            blk = st[:, ((t - 1) * 2 + enc) * 128:((t - 1) * 2 + enc) * 128 + 128]
            half = 0.5 if enc == 0 else 1.0
            if t <= T:
                js = np.arange(NH)
                if t >= 2:
                    blk[js, js] = -bp[t] * half
                    if enc == 0:
                        blk[109, js] += -bp[t] * 0.5
                blk[109, js] += -d[t] + d[t] * Mb
                blk[100:109, 0:100] += d[t] * Mx.T
                if t == 1:
                    blk[109, js] += -1.0
            if t >= 2:
                p = (t - 2) % NSLOT
                for o in range(NO):
                    col = 100 + 2 * p + o
                    blk[0:100, col] = W2d[o, :] * half
                    blk[109, col] = b2d[o] + (0.5 * W2d[o, :].sum() if enc == 0
                                              else 0.0)
    return st.astype(f32)


def _build_nc(sh, w):
    """Build + compile the Bass program for shard size `sh`, round width `w`."""
    import concourse.bass as bass
    import concourse.tile as tile
    from concourse import bacc, mybir

    f32d = mybir.dt.float32
    f32r = mybir.dt.float32r
    Copy = mybir.ActivationFunctionType.Copy
    Sign = mybir.ActivationFunctionType.Sign
    Alu = mybir.AluOpType
    nr = sh // w              # rounds
    nj = sh // 128            # transpose groups
    assert nr % 2 == 0
    # per-engine local column offset of round r inside the c2a/c2b staging
    coff = {e: [0] * (nr + 1) for e in range(2)}
    for e in range(2):
        for r in range(nr):
            coff[e][r + 1] = coff[e][r] + SL[r % 2][e][1]

    nc = bacc.Bacc("TRN2", target_bir_lowering=False, debug=False,
                   num_devices=NCORES)

    xt_d = nc.dram_tensor("xt", [NI + 1, sh], f32d, kind="ExternalInput").ap()
    zz_d = nc.dram_tensor("zz", [128, 1024], f32d, kind="ExternalInput").ap()
    st_d = nc.dram_tensor("st", [128, 52 * 128], f32d,
                          kind="ExternalInput").ap()
    id_d = nc.dram_tensor("ident", [128, 128], f32d, kind="ExternalInput").ap()
    out_d = nc.dram_tensor("out", [T, sh, NO], f32d, kind="ExternalOutput").ap()

    with tile.TileContext(nc) as tc:
        with tc.tile_pool(name="const", bufs=1) as cp, \
             tc.tile_pool(name="stage", bufs=1) as sg:

            ident = cp.tile([128, 128], f32d)
            # one tile per (t, enc) so each matmul waits only its own chunk;
            # DMAs are emitted AFTER the first pair's x feeds (see below) so
            # step 1 is not stuck behind the whole stationary bank.
            st = [cp.tile([128, 128], f32d, name=f"st{k}") for k in range(52)]

            # per (engine, parity) private tiles
            s_t, v_t = {}, {}
            c2a, c2b = {}, {}
            c2r = {}     # raw snapshot-B staging (GPSIMD does the subtract)
            engs = [nc.scalar, nc.vector]
            ps_pool = tc.tile_pool(name="psV", bufs=1,
                                   space=bass.MemorySpace.PSUM)
            ps = ps_pool.__enter__()
            for e, en in enumerate(ENAMES):
                c2a[e] = sg.tile([128, coff[e][nr]], f32d, name=f"c2a_{en}")
                c2b[e] = sg.tile([128, coff[e][nr]], f32d, name=f"c2b_{en}")
                c2r[e] = sg.tile([128, coff[e][nr]], f32d, name=f"c2r_{en}")
                for par in range(2):
                    ew = SL[par][e][1]
                    s_ = sg.tile([128, ew], f32d, name=f"s_{en}{par}")
                    # zero all rows by DMA (memset rejects f32r tiles); the
                    # x-DMA then fills rows 100:110 ([x; ones])
                    nc.sync.dma_start(s_[:], zz_d[:, 0:ew])
                    s_t[(e, par)] = s_
                    v_t[(e, par)] = ps.tile([128, ew], f32d,
                                            name=f"v_{en}{par}")
            # warm the Sign activation table while DMAs stream
            warm_in = sg.tile([1, 8], f32d)
            sgn_warm = sg.tile([1, 8], f32d)
            nc.vector.memset(warm_in[:], 1.0)
            nc.scalar.activation(sgn_warm[:], warm_in[:], Sign)

            for pair in range(nr // 2):
                rounds = (2 * pair, 2 * pair + 1)
                for r in rounds:
                    par = r % 2
                    for e in range(2):
                        eb, ew = SL[par][e]
                        nc.sync.dma_start(
                            s_t[(e, par)][100:110, :],
                            xt_d[:, r * w + eb:r * w + eb + ew])
                if pair == 0:
                    for k in range(52):
                        nc.sync.dma_start(st[k][:],
                                          st_d[:, k * 128:(k + 1) * 128])
                    nc.sync.dma_start(ident[:], id_d[:])
                for t in range(1, T + 2):
                    for r in rounds:
                        par = r % 2
                        for e in range(2):
                            ew = SL[par][e][1]
                            for c0 in range(0, ew, 512):
                                nc.tensor.matmul(
                                    v_t[(e, par)][:, c0:c0 + 512],
                                    st[(t - 1) * 2 + ENC[e]][:],
                                    s_t[(e, par)][:, c0:c0 + 512],
                                    start=(t == 1), stop=True,
                                    skip_group_check=True)
                        if t <= T:
                            nc.scalar.activation(s_t[(0, par)][0:100, :],
                                                 v_t[(0, par)][0:100, :],
                                                 Sign)
                            nc.vector.tensor_scalar(
                                s_t[(1, par)][0:100, :],
                                v_t[(1, par)][0:100, :],
                                0.0, None, Alu.is_gt)
                        if t == 15:
                            # slots (rows 100:128) hold cur2_1..14; each
                            # engine copies its own V tile rows 64:128 (32-
                            # aligned start; stray V rows are harmless)
                            for e in range(2):
                                cs = slice(coff[e][r], coff[e][r + 1])
                                if e == 0:
                                    nc.scalar.activation(
                                        c2a[e][64:128, cs],
                                        v_t[(e, par)][64:128, :],
                                        Copy, bias=0.0, scale=1.0)
                                else:
                                    nc.vector.tensor_copy(
                                        c2a[e][64:128, cs],
                                        v_t[(e, par)][64:128, :])
                        if t == T + 1:
                            # slots 0..10 now cur2_{p+1} + cur2_{p+15}: stage
                            # the raw slot rows (V is about to be reused),
                            # GPSIMD differences them in SBUF off the critical
                            # path.
                            for e in range(2):
                                cs = slice(coff[e][r], coff[e][r + 1])
                                if e == 0:
                                    nc.scalar.activation(
                                        c2r[e][64:128, cs],
                                        v_t[(e, par)][64:128, :],
                                        Copy, bias=0.0, scale=1.0)
                                else:
                                    nc.vector.tensor_copy(
                                        c2r[e][64:128, cs],
                                        v_t[(e, par)][64:128, :])
                                nc.gpsimd.tensor_tensor(
                                    c2b[e][64:128, cs], c2r[e][64:128, cs],
                                    c2a[e][64:128, cs], Alu.subtract)
            ps_pool.__exit__(None, None, None)

            # ---- tail: transpose to batch-major, mem2 recurrence, DMA out --
            # group g (128 device cols) -> engine tile + local columns
            def gmap(g):
                r, off = (g * 128) // w, (g * 128) % w
                for e in range(2):
                    eb, ew = SL[r % 2][e]
                    if eb <= off < eb + ew:
                        return e, coff[e][r] + off - eb
                raise AssertionError

            ps_m2 = tc.tile_pool(name="psM", bufs=1,
                                 space=bass.MemorySpace.PSUM)
            ps2 = ps_m2.__enter__()
            nh = nj // 2
            m2h = [ps2.tile([128, nh, 64], f32d, name="m2d"),
                   ps2.tile([128, nh, 64], f32d, name="m2p")]
            # host ident maps slot row 100+k -> transpose output col k, so
            # the 36 stray V rows in c2a/c2b get zero coefficients
            for g in range(nj):
                e, lc = gmap(g)
                h, jl = (0, g) if g < nh else (1, g - nh)
                nc.tensor.transpose(m2h[h][:, jl, 0:SROW],
                                    c2a[e][64:128, lc:lc + 128],
                                    ident[64:128, 0:SROW])
                nc.tensor.transpose(m2h[h][:, jl, SROW:50],
                                    c2b[e][64:128, lc:lc + 128],
                                    ident[64:128, 0:22])

            def tidx(t):
                return 2 * (t - 1) if t <= 14 else SROW + 2 * (t - 15)

            # stage cur2 into ONE SBUF tile (ScalarE, keeping it single-
            # writer); the whole recurrence then runs as ONE wide DVE chain
            # (GPSIMD lacks the fused scalar_tensor_tensor op).
            m2s = sg.tile([128, nj, 50], f32d, name="m2s")
            nc.scalar.activation(m2s[:, 0:nh], m2h[0][:, :, 0:50], Copy,
                                 bias=0.0, scale=1.0)
            nc.scalar.activation(m2s[:, nh:nj], m2h[1][:, :, 0:50], Copy,
                                 bias=0.0, scale=1.0)

            tmp = sg.tile([128, nj, NO], f32d, name="tmpc")
            mst = sg.tile([128, nj, NO], f32d, name="mstc")
            dst_all = out_d.rearrange("t (p j) o -> p t j o", p=128)
            CH = 5  # DMA chunking over t: per-chunk tiles so the output DMA
            # never blocks the recurrence; the chunk-boundary state is copied
            # to mst so the next chunk's first step doesn't read a DMA'd tile.
            nck = (T + CH - 1) // CH
            och = [sg.tile([128, CH, nj, NO], f32d, name=f"osb{k}")
                   for k in range(nck)]
            for t in range(1, T + 1):
                ci = tidx(t)
                k, tl = (t - 1) // CH, (t - 1) % CH
                cur = m2s[:, :, ci:ci + 2]
                ob = och[k][:, tl]
                if t == 1:
                    nc.vector.tensor_copy(ob, cur)
                else:
                    prev = (mst[:] if tl == 0 else och[k][:, tl - 1])
                    # tmp = H(prev-1) - cur2_t ; m_t = beta*prev - tmp
                    nc.vector.scalar_tensor_tensor(tmp[:], prev, 1.0, cur,
                                                   Alu.is_gt, Alu.subtract)
                    nc.vector.scalar_tensor_tensor(ob, prev, float(BETA),
                                                   tmp[:], Alu.mult,
                                                   Alu.subtract)
                if t % CH == 0 or t == T:
                    t0 = (t - 1) // CH * CH   # chunk start (0-based)
                    if t < T:
                        nc.vector.tensor_copy(mst[:], och[k][:, tl])
                    nc.sync.dma_start(dst_all[:, t0:t, :, :],
                                      och[k][:, 0:t - t0])
            ps_m2.__exit__(None, None, None)

    nc.compile()
    return nc


def _get_nc(sh, rc):
    key = (sh, rc)
    if key not in _CACHE:
        _CACHE[key] = _build_nc(sh, W)
    return _CACHE[key]


def kernel(x, W1, b1, W2, b2):
    global _LAST_RESULT, _LAST_IN_MAPS
    from concourse.bass_utils import run_bass_kernel_spmd

    x = np.ascontiguousarray(x, f32)
    W1 = np.asarray(W1, f32)
    b1 = np.asarray(b1, f32)
    W2 = np.asarray(W2, f32)
    b2 = np.asarray(b2, f32)

    sh = SH
    nc = _get_nc(sh, W)
    st = host_stationaries(W1, b1, W2, b2)
    # shifted identity: maps slot row 100+k to transpose output column k
    ident = np.zeros((128, 128), dtype=f32)
    ident[np.arange(100, 128), np.arange(0, 28)] = 1.0

    # column c of the device layout holds batch element perm[c]; chosen so the
    # output DMA writes 512B-contiguous DRAM chunks per partition.
    cols = np.arange(sh)
    perm = (cols % 128) * (sh // 128) + cols // 128

    in_maps = []
    for i in range(NCORES):
        xs = x[i * sh:(i + 1) * sh]
        xt = np.ones((NI + 1, sh), f32)
        xt[0:NI] = xs[perm].T
        in_maps.append({"xt": xt, "st": st, "ident": ident,
                        "zz": np.zeros((128, 1024), f32)})

    _LAST_IN_MAPS = in_maps
    res = run_bass_kernel_spmd(nc, in_maps, list(range(NCORES)))
    _LAST_RESULT = res
    return np.concatenate([res.results[i]["out"] for i in range(NCORES)],
                          axis=1)


# revision 62
# speedup vs baseline: 33.2569x; 33.2569x over previous
"""Trainium2 Bass kernel for the SNN (snntorch Leaky, subtract-reset) forward.

Reference computation:
    cur1 = x @ W1.T + b1                      # [B, 100], static across steps
    25 steps of:  mem1 = 0.95*mem1 + cur1 - H(prev mem1 - 1)
                  spk1 = H(mem1 - 1);  cur2 = spk1 @ W2.T + b2
                  mem2 = 0.95*mem2 + cur2 - H(prev mem2 - 1)
    returns mem2 per step: [25, B, 2]

Device algorithm (per core, batch shard 8192, hidden on partitions):
  Let A = cur1/(1-beta), z = mem1 - A, theta = 1 - A.  Track
      V_t = beta^-t * (z_t - theta)
  so the spike test is the CONSTANT-threshold compare  spk_t = H(V_t):
      V_0 = -1 (uniform);  V_t = V_{t-1} - beta^-t spk_{t-1} - d_t theta,
      d_t = beta^-t - beta^-(t-1).
  The drift d_t*theta = d_t - d_t*(Mx x + Mb) is linear in x (9 features), so
  ONE accumulating PSUM matmul per step applies spike-subtract AND drift AND
  the fc2 readout: moving rows = [spk(100); x_hi(9); ones; x_lo(9); pad],
  stationary [128, 128] = 100 V cols + 28 cur2-slot cols.  Matmuls run as
  bf16 hi/lo pairs (exact products, fp32 PSUM accumulate): 2 cycles/column
  vs fp32's 4, while float32r's reduced precision flips ~3% of spikes on
  real HW.  Spikes come from V via ScalarE Sign (+-1 coded stationaries)
  and DVE is_gt (0/1 coded) on private per-(engine, parity) tiles -- the
  tile framework serializes cross-engine access to a shared tile even on
  disjoint ranges, and GPSIMD cannot touch PSUM at all (walrus ISA rule).
  cur2_t accumulates into a rotating set of 14 PSUM row-pairs; snapshots
  after step 15/26 + SBUF differencing (GPSIMD) recover all 25, then
  PE-transpose to batch-major and a fused 2-op/step DVE recurrence emits
  mem2 through chunked, double-buffered output DMAs.
"""

import numpy as np

BETA = 0.95
T = 25
NI, NH, NO = 9, 100, 2
B = 65536
NCORES = 8
SH = B // NCORES          # batch shard per core
f32 = np.float32

W = 2048                  # round width (columns per engine-group of tiles)
NSLOT = 14                # rotating cur2 row-pair slots
SROW = 28                 # first spike row / number of slot rows
ENAMES = ("act", "dve")
ENC = (0, 1)              # stationary coding per engine: 0 = sign, 1 = 0/1
# per-parity engine slices (base, width) -- PSUM tiles are bank (512 col)
# quantized.  GPSIMD cannot touch PSUM (walrus rule), so only ScalarE and
# DVE read V; GPSIMD gets the SBUF-side work (staging subtract, mem2 half).
SL = [[(0, 1024), (1024, 1024)],
      [(0, 1024), (1024, 1024)]]

_CACHE = {}
_LAST_RESULT = None       # test.py pokes at these for its timing harness
_LAST_IN_MAPS = None


def _build_nop_nc():
    """Minimal kernel (one tiny DMA round-trip) for dispatch-overhead baseline."""
    import concourse.bass as bass
    import concourse.tile as tile
    from concourse import bacc, mybir
    f32d = mybir.dt.float32
    nc = bacc.Bacc("TRN2", target_bir_lowering=False, debug=False,
                   num_devices=NCORES)
    i_d = nc.dram_tensor("nin", [1, 128], f32d, kind="ExternalInput").ap()
    o_d = nc.dram_tensor("nout", [1, 128], f32d, kind="ExternalOutput").ap()
    with tile.TileContext(nc) as tc:
        with tc.tile_pool(name="sb", bufs=1) as sb:
            tl = sb.tile([1, 128], f32d)
            nc.sync.dma_start(tl[:], i_d[:])
            nc.sync.dma_start(o_d[:], tl[:])
    nc.compile()
    return nc


def host_stationaries(W1, b1, W2, b2):
    """[128, 52*128] f32 stationary bank: index (t-1)*2+enc, t = 1..26.

    Rows: 0:100 = spikes, 100:109 = x features, 109 = ones, 110:128 = pad.
    Cols: 0:100 V columns, 100:128 cur2 slots (col 100+2p+o).
    """
    inv = 1.0 / (1.0 - np.float64(BETA))
    Mx = W1.astype(np.float64) * inv          # [100, 9]
    Mb = b1.astype(np.float64) * inv          # [100]
    W2d = W2.astype(np.float64)
    b2d = b2.astype(np.float64)
    bi = 1.0 / np.float64(BETA)
    bp = {t: bi ** t for t in range(0, T + 2)}
    d = {t: bp[t] - bp[t - 1] for t in range(1, T + 2)}

    st = np.zeros((128, 52 * 128), np.float64)
    for t in range(1, T + 2):
        for enc in (0, 1):
            blk = st[:, ((t - 1) * 2 + enc) * 128:((t - 1) * 2 + enc) * 128 + 128]
            half = 0.5 if enc == 0 else 1.0
            if t <= T:
                js = np.arange(NH)
                if t >= 2:
                    blk[js, js] = -bp[t] * half
                    if enc == 0:
                        blk[109, js] += -bp[t] * 0.5
                blk[109, js] += -d[t] + d[t] * Mb
                blk[100:109, 0:100] += d[t] * Mx.T
                blk[110:119, 0:100] += d[t] * Mx.T
                if t == 1:
                    blk[109, js] += -1.0
            if t >= 2:
                p = (t - 2) % NSLOT
                for o in range(NO):
                    col = 100 + 2 * p + o
                    blk[0:100, col] = W2d[o, :] * half
                    blk[109, col] = b2d[o] + (0.5 * W2d[o, :].sum() if enc == 0
                                              else 0.0)
    import ml_dtypes
    bf = ml_dtypes.bfloat16
    st_hi = st.astype(bf)
    st_lo = (st - st_hi.astype(np.float64)).astype(bf)
    return st_hi, st_lo


def _build_nc(sh, w):
    """Build + compile the Bass program for shard size `sh`, round width `w`."""
    import concourse.bass as bass
    import concourse.tile as tile
    from concourse import bacc, mybir

    f32d = mybir.dt.float32
    f32r = mybir.dt.float32r
    Copy = mybir.ActivationFunctionType.Copy
    Sign = mybir.ActivationFunctionType.Sign
    Alu = mybir.AluOpType
    nr = sh // w              # rounds
    nj = sh // 128            # transpose groups
    assert nr % 2 == 0
    # per-engine local column offset of round r inside the c2a/c2b staging
    coff = {e: [0] * (nr + 1) for e in range(2)}
    for e in range(2):
        for r in range(nr):
            coff[e][r + 1] = coff[e][r] + SL[r % 2][e][1]

    nc = bacc.Bacc("TRN2", target_bir_lowering=False, debug=False,
                   num_devices=NCORES)

    bf16 = mybir.dt.bfloat16
    xt_d = nc.dram_tensor("xt", [2 * NI + 1, sh], bf16,
                          kind="ExternalInput").ap()
    sth_d = nc.dram_tensor("sth", [128, 52 * 128], bf16,
                           kind="ExternalInput").ap()
    stl_d = nc.dram_tensor("stl", [128, 52 * 128], bf16,
                           kind="ExternalInput").ap()
    id_d = nc.dram_tensor("ident", [128, 128], f32d, kind="ExternalInput").ap()
    out_d = nc.dram_tensor("out", [T, sh, NO], f32d, kind="ExternalOutput").ap()

    with tile.TileContext(nc) as tc:
        with tc.tile_pool(name="const", bufs=1) as cp, \
             tc.tile_pool(name="stage", bufs=1) as sg:

            ident = cp.tile([128, 128], f32d)
            # one tile per (t, enc) so each matmul waits only its own chunk;
            # DMAs are emitted AFTER the first pair's x feeds (see below) so
            # step 1 is not stuck behind the whole stationary bank.
            st = [[cp.tile([128, 128], bf16, name=f"st{hl}_{k}")
                   for k in range(52)] for hl in range(2)]

            # per (engine, parity) private tiles
            s_t, v_t = {}, {}
            c2a, c2b = {}, {}
            c2r = {}     # raw snapshot-B staging (GPSIMD does the subtract)
            engs = [nc.scalar, nc.vector]
            ps_pool = tc.tile_pool(name="psV", bufs=1,
                                   space=bass.MemorySpace.PSUM)
            ps = ps_pool.__enter__()
            for e, en in enumerate(ENAMES):
                c2a[e] = sg.tile([128, coff[e][nr]], f32d, name=f"c2a_{en}")
                c2b[e] = sg.tile([128, coff[e][nr]], f32d, name=f"c2b_{en}")
                c2r[e] = sg.tile([128, coff[e][nr]], f32d, name=f"c2r_{en}")
                for par in range(2):
                    ew = SL[par][e][1]
                    s_ = sg.tile([128, ew], bf16, name=f"s_{en}{par}")
                    # zero spike/pad rows (bf16 memset is ISA-legal); the
                    # x-DMA then fills rows 100:119 ([x_hi; ones; x_lo])
                    nc.vector.memset(s_[0:64, :], 0.0)
                    nc.gpsimd.memset(s_[64:128, :], 0.0)
                    s_t[(e, par)] = s_
                    v_t[(e, par)] = ps.tile([128, ew], f32d,
                                            name=f"v_{en}{par}")
            # warm the Sign activation table while DMAs stream
            warm_in = sg.tile([1, 8], f32d)
            sgn_warm = sg.tile([1, 8], f32d)
            nc.vector.memset(warm_in[:], 1.0)
            nc.scalar.activation(sgn_warm[:], warm_in[:], Sign)

            for pair in range(nr // 2):
                rounds = (2 * pair, 2 * pair + 1)
                for r in rounds:
                    par = r % 2
                    for e in range(2):
                        eb, ew = SL[par][e]
                        nc.sync.dma_start(
                            s_t[(e, par)][100:119, :],
                            xt_d[:, r * w + eb:r * w + eb + ew])
                if pair == 0:
                    for k in range(52):
                        nc.sync.dma_start(st[0][k][:],
                                          sth_d[:, k * 128:(k + 1) * 128])
                        nc.sync.dma_start(st[1][k][:],
                                          stl_d[:, k * 128:(k + 1) * 128])
                    nc.sync.dma_start(ident[:], id_d[:])
                for t in range(1, T + 2):
                    for r in rounds:
                        par = r % 2
                        for e in range(2):
                            ew = SL[par][e][1]
                            nlimb = 1 if t == T + 1 else 2
                            for c0 in range(0, ew, 512):
                                for hl in range(nlimb):
                                    nc.tensor.matmul(
                                        v_t[(e, par)][:, c0:c0 + 512],
                                        st[hl][(t - 1) * 2 + ENC[e]][:],
                                        s_t[(e, par)][:, c0:c0 + 512],
                                        start=(t == 1 and hl == 0), stop=True,
                                        skip_group_check=True)
                        if t <= T:
                            nc.scalar.activation(s_t[(0, par)][0:100, :],
                                                 v_t[(0, par)][0:100, :],
                                                 Sign)
                            nc.vector.tensor_scalar(
                                s_t[(1, par)][0:100, :],
                                v_t[(1, par)][0:100, :],
                                0.0, None, Alu.is_gt)
                        if t == 15:
                            # slots (rows 100:128) hold cur2_1..14; each
                            # engine copies its own V tile rows 64:128 (32-
                            # aligned start; stray V rows are harmless)
                            for e in range(2):
                                cs = slice(coff[e][r], coff[e][r + 1])
                                if e == 0:
                                    nc.scalar.activation(
                                        c2a[e][64:128, cs],
                                        v_t[(e, par)][64:128, :],
                                        Copy, bias=0.0, scale=1.0)
                                else:
                                    nc.vector.tensor_copy(
                                        c2a[e][64:128, cs],
                                        v_t[(e, par)][64:128, :])
                        if t == T + 1:
                            # slots 0..10 now cur2_{p+1} + cur2_{p+15}: stage
                            # the raw slot rows (V is about to be reused),
                            # GPSIMD differences them in SBUF off the critical
                            # path.
                            for e in range(2):
                                cs = slice(coff[e][r], coff[e][r + 1])
                                if e == 0:
                                    nc.scalar.activation(
                                        c2r[e][64:128, cs],
                                        v_t[(e, par)][64:128, :],
                                        Copy, bias=0.0, scale=1.0)
                                else:
                                    nc.vector.tensor_copy(
                                        c2r[e][64:128, cs],
                                        v_t[(e, par)][64:128, :])
                                deng = nc.gpsimd if e == 0 else nc.vector
                                deng.tensor_tensor(
                                    c2b[e][64:128, cs], c2r[e][64:128, cs],
                                    c2a[e][64:128, cs], Alu.subtract)
            ps_pool.__exit__(None, None, None)

            # ---- tail: transpose to batch-major, mem2 recurrence, DMA out --
            # group g (128 device cols) -> engine tile + local columns
            def gmap(g):
                r, off = (g * 128) // w, (g * 128) % w
                for e in range(2):
                    eb, ew = SL[r % 2][e]
                    if eb <= off < eb + ew:
                        return e, coff[e][r] + off - eb
                raise AssertionError

            ps_m2 = tc.tile_pool(name="psM", bufs=1,
                                 space=bass.MemorySpace.PSUM)
            ps2 = ps_m2.__enter__()
            nh = nj // 2
            m2h = [ps2.tile([128, nh, 64], f32d, name="m2d"),
                   ps2.tile([128, nh, 64], f32d, name="m2p")]
            # host ident maps slot row 100+k -> transpose output col k, so
            # the 36 stray V rows in c2a/c2b get zero coefficients
            for g in range(nj):
                e, lc = gmap(g)
                h, jl = (0, g) if g < nh else (1, g - nh)
                nc.tensor.transpose(m2h[h][:, jl, 0:SROW],
                                    c2a[e][64:128, lc:lc + 128],
                                    ident[64:128, 0:SROW])

            def tidx(t):
                return 2 * (t - 1) if t <= 14 else SROW + 2 * (t - 15)

            # stage cur2 into ONE SBUF tile (ScalarE, keeping it single-
            # writer); the whole recurrence then runs as ONE wide DVE chain
            # (GPSIMD lacks the fused scalar_tensor_tensor op).
            m2s = sg.tile([128, nj, 50], f32d, name="m2s")
            nc.scalar.activation(m2s[:, 0:nh, 0:SROW],
                                 m2h[0][:, :, 0:SROW], Copy,
                                 bias=0.0, scale=1.0)
            nc.scalar.activation(m2s[:, nh:nj, 0:SROW],
                                 m2h[1][:, :, 0:SROW], Copy,
                                 bias=0.0, scale=1.0)

            tmp = sg.tile([128, nj, NO], f32d, name="tmpc")
            mst = sg.tile([128, nj, NO], f32d, name="mstc")
            dst_all = out_d.rearrange("t (p j) o -> p t j o", p=128)
            CH = 5  # DMA chunking over t: per-chunk tiles so the output DMA
            # never blocks the recurrence; the chunk-boundary state is copied
            # to mst so the next chunk's first step doesn't read a DMA'd tile.
            nck = (T + CH - 1) // CH
            och = [sg.tile([128, CH, nj, NO], f32d, name=f"osb{k}")
                   for k in range(nck)]
            for t in range(1, T + 1):
                if t == 15:
                    # steps 1..14 only need snapshot-A data; the B transposes
                    # and second staging piece overlap that part of the chain
                    for g in range(nj):
                        e, lc = gmap(g)
                        h, jl = (0, g) if g < nh else (1, g - nh)
                        nc.tensor.transpose(m2h[h][:, jl, SROW:50],
                                            c2b[e][64:128, lc:lc + 128],
                                            ident[64:128, 0:22])
                    nc.scalar.activation(m2s[:, 0:nh, SROW:50],
                                         m2h[0][:, :, SROW:50], Copy,
                                         bias=0.0, scale=1.0)
                    nc.scalar.activation(m2s[:, nh:nj, SROW:50],
                                         m2h[1][:, :, SROW:50], Copy,
                                         bias=0.0, scale=1.0)
                ci = tidx(t)
                k, tl = (t - 1) // CH, (t - 1) % CH
                cur = m2s[:, :, ci:ci + 2]
                ob = och[k][:, tl]
                if t == 1:
                    nc.vector.tensor_copy(ob, cur)
                else:
                    prev = (mst[:] if tl == 0 else och[k][:, tl - 1])
                    # tmp = H(prev-1) - cur2_t ; m_t = beta*prev - tmp
                    nc.vector.scalar_tensor_tensor(tmp[:], prev, 1.0, cur,
                                                   Alu.is_gt, Alu.subtract)
                    nc.vector.scalar_tensor_tensor(ob, prev, float(BETA),
                                                   tmp[:], Alu.mult,
                                                   Alu.subtract)
                if t % CH == 0 or t == T:
                    t0 = (t - 1) // CH * CH   # chunk start (0-based)
                    if t < T:
                        nc.vector.tensor_copy(mst[:], och[k][:, tl])
                    nc.sync.dma_start(dst_all[:, t0:t, :, :],
                                      och[k][:, 0:t - t0])
            ps_m2.__exit__(None, None, None)

    nc.compile()
    return nc


def _get_nc(sh, rc):
    key = (sh, rc)
    if key not in _CACHE:
        _CACHE[key] = _build_nc(sh, W)
    return _CACHE[key]


def kernel(x, W1, b1, W2, b2):
    global _LAST_RESULT, _LAST_IN_MAPS
    from concourse.bass_utils import run_bass_kernel_spmd

    x = np.ascontiguousarray(x, f32)
    W1 = np.asarray(W1, f32)
    b1 = np.asarray(b1, f32)
    W2 = np.asarray(W2, f32)
    b2 = np.asarray(b2, f32)

    import ml_dtypes
    bf = ml_dtypes.bfloat16
    sh = SH
    nc = _get_nc(sh, W)
    st_hi, st_lo = host_stationaries(W1, b1, W2, b2)
    # shifted identity: maps slot row 100+k to transpose output column k
    ident = np.zeros((128, 128), dtype=f32)
    ident[np.arange(100, 128), np.arange(0, 28)] = 1.0

    # column c of the device layout holds batch element perm[c]; chosen so the
    # output DMA writes 512B-contiguous DRAM chunks per partition.
    cols = np.arange(sh)
    perm = (cols % 128) * (sh // 128) + cols // 128

    in_maps = []
    for i in range(NCORES):
        xs = x[i * sh:(i + 1) * sh]
        xt = np.ones((2 * NI + 1, sh), bf)
        xT = xs[perm].T.astype(np.float64)
        x_hi = xT.astype(bf)
        xt[0:NI] = x_hi
        xt[NI + 1:2 * NI + 1] = (xT - x_hi.astype(np.float64)).astype(bf)
        in_maps.append({"xt": xt, "sth": st_hi, "stl": st_lo,
                        "ident": ident})

    _LAST_IN_MAPS = in_maps
    res = run_bass_kernel_spmd(nc, in_maps, list(range(NCORES)))
    _LAST_RESULT = res
    return np.concatenate([res.results[i]["out"] for i in range(NCORES)],
                          axis=1)
